# revision 1
# baseline (speedup 1.0000x reference)
"""Trainium2 Bass kernel for nn_BinRegularizer (histogram_binning).

Replicates the reference's sequential-f32 segment_sum numerics:
- per-chunk (2048-element) fused compute+reduce passes on DVE + ACT produce
  counts, relu sums, and rne-quantized cumulative sums on dyadic grids
- the host replays the sequential-f32 accumulation trajectory per bin at
  chunk granularity (while the running partial sits in a binade with ulp u,
  each element contributes u*rne(x/u) exactly), then computes the 5 outputs.

Self-contained: hardcodes shapes (4096x16384 f32 weights, alpha[1]),
8 NeuronCores, sharding = contiguous 8M-element blocks per core.
"""
import sys

sys.path.insert(0, "/opt/trn_rl_repo")

import numpy as np

f32 = np.float32

P = 128          # partitions
F = 2048         # free dim per tile = chunk size
NT = 32          # tiles per core
NCORES = 8
CORE_ELEMS = P * F * NT          # 8M
N_TOTAL = CORE_ELEMS * NCORES    # 64M
NCHUNK = NCORES * NT * P         # 32768 chunks of 2048, stream order

# dyadic grids measured on device
US_S = [2.0**-7, 2.0**-6, 2.0**-5, 2.0**-4, 2.0**-3, 2.0**-2]
AV_S = {0: US_S[:5], 1: US_S[:5], 3: US_S}
US_Q = [2.0**-12, 2.0**-11, 2.0**-10, 2.0**-9, 2.0**-8, 2.0**-7, 2.0**-6, 2.0**-5]
AV_Q = {0: [2.0**-9, 2.0**-8, 2.0**-7, 2.0**-6],
        1: [2.0**-10, 2.0**-9, 2.0**-8, 2.0**-7],
        2: [2.0**-12, 2.0**-11, 2.0**-10],
        3: [2.0**-10, 2.0**-9, 2.0**-8, 2.0**-7, 2.0**-6, 2.0**-5]}
# relu-type quantities computed on DVE (stt with zeros) instead of ACT;
# chosen for engine balance
DVE_RELUS = {
    "qLm2@-12", "qL0@-12", "qLp2@-12",
    "qLm2@-11", "qL0@-11", "qLp2@-11",
    "qLm1@-10", "qLm2@-10", "qL0@-10", "qLp2@-10",
    "sL1@-7", "sL2@-7", "sR3@-7",
    "sL1@-6", "sL2@-6",
}

NC_SLOTS = 96

_qneed = {}
for _k, _us in AV_Q.items():
    for _u in _us:
        _qneed.setdefault(int(np.log2(_u)), set()).add(_k)


def MS(u):
    return f32(f32(3.0 * 2.0**22) * f32(u))


def _qz_of(x, u):
    m = MS(u)
    return f32(f32(f32(x) + m) - m)


def _const_values(a):
    """slot-name -> f32 value. Shared vocabulary with the builder."""
    th1 = f32(f32(-1.5) * a)
    th2 = f32(f32(-0.5) * a)
    th3 = f32(f32(0.5) * a)
    tau1 = f32(th1 * th1)
    tau2 = f32(th3 * th3)
    lv0 = f32(f32(-2) * a)
    lv1 = f32(f32(-1) * a)
    lv3 = f32(f32(1) * a)
    vals = {
        "th1": th1, "th2": th2, "th3": th3,
        "nth1": f32(-th1), "nth2": f32(-th2), "nth3": f32(-th3),
        "nm2a": f32(-lv0), "nm1a": f32(-lv1), "np1a": f32(-lv3),
    }
    for u in US_S:
        lg = int(np.log2(u))
        m = MS(u)
        vals[f"st1@{lg}"] = f32(m + _qz_of(th1, u))
        vals[f"st2@{lg}"] = f32(m + _qz_of(th2, u))
        vals[f"st3@{lg}"] = f32(m + _qz_of(th3, u))
        vals[f"nst3@{lg}"] = f32(-vals[f"st3@{lg}"])
    for u in US_Q:
        lg = int(np.log2(u))
        m = MS(u)
        vals[f"qm1@{lg}"] = f32(m - _qz_of(tau1, u))
        vals[f"qm2@{lg}"] = f32(m - _qz_of(tau2, u))
        vals[f"qz0@{lg}"] = m
        vals[f"qp2@{lg}"] = f32(m + _qz_of(tau2, u))
        vals[f"nqp2@{lg}"] = f32(-vals[f"qp2@{lg}"])
    return vals


def _s_names(u):
    return ["sR3"] if u == US_S[-1] else ["sL1", "sL2", "sR3"]


def _q_names(lg):
    ks = _qneed[lg]
    names = []
    if 0 in ks or 1 in ks:
        names.append(("qLm1", f"qm1@{lg}", True))
    if 1 in ks or 2 in ks:
        names.append(("qLm2", f"qm2@{lg}", True))
    if 2 in ks:
        names.append(("qL0", f"qz0@{lg}", True))
        names.append(("qLp2", f"qp2@{lg}", True))
    if 3 in ks:
        names.append(("qRp2", f"qp2@{lg}", False))
    return names


_CACHE = {}


def _build_program():
    import concourse.bacc as bacc
    import concourse.tile as tile
    from concourse import mybir

    AL = mybir.AluOpType
    AF = mybir.ActivationFunctionType
    DT = mybir.dt.float32

    cslot = {}

    def slot(name):
        if name not in cslot:
            cslot[name] = len(cslot)
        return cslot[name]

    # ---- stats layouts ----
    layout_d = {}
    layout_a = {}

    def alloc_stat(name, eng):
        lay = layout_d if eng == "d" else layout_a
        lay[name] = len(lay)

    for nm in ("C1", "C2", "C3", "T1", "E1", "E2", "E3"):
        alloc_stat(nm, "d")
    for nm in ("Bm2a", "B1", "Bm1a", "B2", "B0", "B3", "B1a", "T2"):
        alloc_stat(nm, "a")
    for u in US_S:
        lg = int(np.log2(u))
        for nm in _s_names(u):
            full = f"{nm}@{lg}"
            alloc_stat(full, "d" if full in DVE_RELUS else "a")
    for u in US_Q:
        lg = int(np.log2(u))
        for nm, _cn, _lo in _q_names(lg):
            full = f"{nm}@{lg}"
            alloc_stat(full, "d" if full in DVE_RELUS else "a")
    NQ_D = len(layout_d)
    NQ_A = len(layout_a)

    nc = bacc.Bacc("TRN2", target_bir_lowering=False, debug=False,
                   num_devices=NCORES)
    W = nc.dram_tensor("w", [NT * P, F], DT, kind="ExternalInput")
    CONST = nc.dram_tensor("consts", [P, NC_SLOTS], DT, kind="ExternalInput")
    OUTD = nc.dram_tensor("outd", [P, NQ_D * NT], DT, kind="ExternalOutput")
    OUTA = nc.dram_tensor("outa", [P, NQ_A * NT], DT, kind="ExternalOutput")
    Wv = W[:, :].rearrange("(t p) f -> t p f", p=P)

    with tile.TileContext(nc) as tc:
        with tc.tile_pool(name="wp", bufs=3) as wpool, \
             tc.tile_pool(name="yp", bufs=2) as ypool, \
             tc.tile_pool(name="y2p", bufs=2) as y2pool, \
             tc.tile_pool(name="rp", bufs=4) as rpool, \
             tc.tile_pool(name="zp", bufs=2) as zpool, \
             tc.tile_pool(name="singles", bufs=1) as singles:
            cd = singles.tile([P, NC_SLOTS], DT)
            ca = singles.tile([P, NC_SLOTS], DT)
            zeros = singles.tile([P, F], DT)
            std = singles.tile([P, NQ_D * NT], DT)
            sta = singles.tile([P, NQ_A * NT], DT)
            gd = singles.tile([P, F], DT)
            ga = singles.tile([P, F], DT)

            nc.sync.dma_start(out=cd, in_=CONST[:, :])
            nc.scalar.copy(out=ca, in_=cd)
            nc.vector.memset(zeros, 0.0)

            def cs_d(nm):
                i = slot(nm)
                return cd[:, i:i + 1]

            def cs_a(nm):
                i = slot(nm)
                return ca[:, i:i + 1]

            def st(name, t):
                if name in layout_d:
                    q = layout_d[name]
                    return std[:, q * NT + t:q * NT + t + 1]
                q = layout_a[name]
                return sta[:, q * NT + t:q * NT + t + 1]

            for t in range(NT):
                w = wpool.tile([P, F], DT, tag="w")
                nc.sync.dma_start(out=w, in_=Wv[t])

                # ---- phase A on DVE: counts + T1 ----
                for nm, cn, op in (("C1", "th1", AL.is_gt),
                                   ("C2", "th2", AL.is_ge),
                                   ("C3", "th3", AL.is_gt)):
                    nc.vector.tensor_scalar(
                        out=gd[:, :], in0=w[:, :], scalar1=cs_d(cn),
                        scalar2=None, op0=op, op1=AL.add,
                        accum_out=st(nm, t))
                nc.vector.tensor_scalar(
                    out=gd[:, :], in0=w[:, :], scalar1=0.0,
                    scalar2=None, op0=AL.add, op1=AL.add,
                    accum_out=st("T1", t))

                # ---- phase A on ACT: B relus, T2, |w| ----
                rtiles = {}
                for nm, cn in (("Bm2a", "nm2a"), ("B1", "nth1"),
                               ("Bm1a", "nm1a"), ("B2", "nth2"),
                               ("B0", None), ("B3", "nth3"),
                               ("B1a", "np1a")):
                    if nm in ("B1", "B2", "B3"):
                        rout = rpool.tile([P, F], DT, tag="r")
                        rtiles[nm] = rout
                    else:
                        rout = ga
                    nc.scalar.activation(
                        out=rout[:, :], in_=w[:, :], func=AF.Relu,
                        bias=(0.0 if cn is None else cs_a(cn)), scale=1.0,
                        accum_out=st(nm, t))
                absw = zpool.tile([P, F], DT, tag="absw")
                nc.scalar.activation(out=absw[:, :], in_=w[:, :], func=AF.Abs,
                                     bias=0.0, scale=1.0)
                nc.scalar.activation(out=ga[:, :], in_=w[:, :], func=AF.Square,
                                     bias=0.0, scale=1.0, accum_out=st("T2", t))

                # ---- E sums on DVE ----
                for j, nm in enumerate(("B1", "B2", "B3")):
                    nc.vector.scalar_tensor_tensor(
                        out=gd[:, :], in0=rtiles[nm][:, :], scalar=1.0,
                        in1=rtiles[nm][:, :], op0=AL.mult, op1=AL.mult,
                        accum_out=st(f"E{j + 1}", t))

                # ---- signed square z = w * |w| ----
                z = zpool.tile([P, F], DT, tag="z")
                nc.vector.tensor_mul(out=z[:, :], in0=w[:, :], in1=absw[:, :])

                def emit_cum(full, src_tile, cn, lower, t):
                    if full in DVE_RELUS:
                        nc.vector.scalar_tensor_tensor(
                            out=gd[:, :], in0=src_tile[:, :], scalar=cs_d(cn),
                            in1=zeros[:, :], op0=AL.subtract,
                            op1=(AL.min if lower else AL.max),
                            accum_out=st(full, t))
                    elif lower:
                        nc.scalar.activation(
                            out=ga[:, :], in_=src_tile[:, :], func=AF.Relu,
                            bias=cs_a(cn), scale=-1.0, accum_out=st(full, t))
                    else:
                        nc.scalar.activation(
                            out=ga[:, :], in_=src_tile[:, :], func=AF.Relu,
                            bias=cs_a("n" + cn), scale=1.0,
                            accum_out=st(full, t))

                # ---- s-side quantized passes ----
                for u in US_S:
                    lg = int(np.log2(u))
                    y = ypool.tile([P, F], DT, tag="y")
                    nc.vector.tensor_scalar(
                        out=y[:, :], in0=w[:, :], scalar1=float(MS(u)),
                        scalar2=None, op0=AL.add)
                    for nm in _s_names(u):
                        cn = {"sL1": f"st1@{lg}", "sL2": f"st2@{lg}",
                              "sR3": f"st3@{lg}"}[nm]
                        emit_cum(f"{nm}@{lg}", y, cn, nm.startswith("sL"), t)

                # ---- sq-side quantized passes (z-space) ----
                for u in US_Q:
                    lg = int(np.log2(u))
                    y2 = y2pool.tile([P, F], DT, tag="y2")
                    nc.vector.tensor_scalar(
                        out=y2[:, :], in0=z[:, :], scalar1=float(MS(u)),
                        scalar2=None, op0=AL.add)
                    for nm, cn, lower in _q_names(lg):
                        emit_cum(f"{nm}@{lg}", y2, cn, lower, t)

            nc.sync.dma_start(out=OUTD[:, :], in_=std)
            nc.sync.dma_start(out=OUTA[:, :], in_=sta)

    assert len(cslot) <= NC_SLOTS, len(cslot)
    nc.compile()
    return nc, cslot, layout_d, layout_a, NQ_D, NQ_A


def _get_program():
    if "prog" not in _CACHE:
        _CACHE["prog"] = _build_program()
    return _CACHE["prog"]


def kernel(weights, alpha):
    from concourse.bass_utils import run_bass_kernel_spmd

    w_full = np.ascontiguousarray(weights, dtype=np.float32).reshape(-1)
    a = f32(np.asarray(alpha, dtype=np.float32).reshape(-1)[0])
    assert w_full.size == N_TOTAL

    nc, cslot, layout_d, layout_a, NQ_D, NQ_A = _get_program()

    vals = _const_values(a)
    cvals = np.zeros(NC_SLOTS, f32)
    for nm, i in cslot.items():
        cvals[i] = vals[nm]
    consts_np = np.ascontiguousarray(np.broadcast_to(cvals, (P, NC_SLOTS)))

    in_maps = []
    for c in range(NCORES):
        shard = w_full[c * CORE_ELEMS:(c + 1) * CORE_ELEMS].reshape(NT * P, F)
        in_maps.append({"w": shard, "consts": consts_np})

    res = run_bass_kernel_spmd(nc, in_maps, core_ids=list(range(NCORES)))

    tabs = {}
    for lay, key, nq in ((layout_d, "outd", NQ_D), (layout_a, "outa", NQ_A)):
        for qname, qi in lay.items():
            arr = np.empty(NCHUNK, np.float64)
            for c in range(NCORES):
                block = res.results[c][key].reshape(P, nq, NT)
                arr[c * NT * P:(c + 1) * NT * P] = block[:, qi, :].T.reshape(-1)
            tabs[qname] = arr

    return _finish(tabs, a)


def _finish(dev, a):
    th1 = f32(f32(-1.5) * a)
    th2 = f32(f32(-0.5) * a)
    th3 = f32(f32(0.5) * a)
    tau1 = f32(th1 * th1)
    tau2 = f32(th3 * th3)
    lv = [f32(f32(-2) * a), f32(f32(-1) * a), f32(0.0), f32(f32(1) * a)]

    nvec = np.full(NCHUNK, float(F))
    C1, C2, C3 = dev["C1"], dev["C2"], dev["C3"]

    S_gt1 = dev["B1"] + float(th1) * C1
    S_ge2 = dev["B2"] + float(th2) * C2
    S_gt3 = dev["B3"] + float(th3) * C3
    s_ab = np.stack([dev["T1"] - S_gt1, S_gt1 - S_ge2, S_ge2 - S_gt3, S_gt3], 1)
    Q_gt1 = dev["E1"] + 2 * float(th1) * dev["B1"] + float(th1)**2 * C1
    Q_ge2 = dev["E2"] + 2 * float(th2) * dev["B2"] + float(th2)**2 * C2
    Q_gt3 = dev["E3"] + 2 * float(th3) * dev["B3"] + float(th3)**2 * C3
    sq_ab = np.stack([dev["T2"] - Q_gt1, Q_gt1 - Q_ge2, Q_ge2 - Q_gt3, Q_gt3], 1)
    cnt_ab = np.stack([nvec - C1, C1 - C2, C2 - C3, C3], 1)

    def getL(name):
        v = dev[name]
        return -v if name in DVE_RELUS else v

    sig_s = {}
    for u in US_S:
        lg = int(np.log2(u))
        qt1 = float(_qz_of(th1, u))
        qt2 = float(_qz_of(th2, u))
        qt3 = float(_qz_of(th3, u))
        G3 = dev[f"sR3@{lg}"] + qt3 * C3
        if u == US_S[-1]:
            zz = np.zeros(NCHUNK)
            sig_s[u] = np.stack([zz, zz, zz, G3], 1)
        else:
            F1 = qt1 * (nvec - C1) - getL(f"sL1@{lg}")
            F2p = qt2 * (nvec - C2) - getL(f"sL2@{lg}")
            sig_s[u] = np.stack([F1, F2p - F1, np.zeros(NCHUNK), G3], 1)

    sig_q = {}
    for u in US_Q:
        lg = int(np.log2(u))
        ks = _qneed[lg]
        qm1 = float(-_qz_of(tau1, u))
        qm2 = float(-_qz_of(tau2, u))
        qp2 = float(_qz_of(tau2, u))
        cols = [np.zeros(NCHUNK) for _ in range(4)]
        Sm1 = Sm2 = S0 = Sp2 = None
        if 0 in ks or 1 in ks:
            Sm1 = qm1 * (nvec - C1) - getL(f"qLm1@{lg}")
        if 1 in ks or 2 in ks:
            Sm2 = qm2 * (nvec - C2) - getL(f"qLm2@{lg}")
        if 2 in ks:
            S0 = -getL(f"qL0@{lg}")
            Sp2 = qp2 * (nvec - C3) - getL(f"qLp2@{lg}")
        if 0 in ks:
            cols[0] = -Sm1
        if 1 in ks:
            cols[1] = -(Sm2 - Sm1)
        if 2 in ks:
            cols[2] = (Sp2 - S0) - (S0 - Sm2)
        if 3 in ks:
            cols[3] = dev[f"qRp2@{lg}"] + qp2 * C3
        sig_q[u] = np.stack(cols, 1)

    def replay(k, tabs, fallback, avail):
        avail = sorted(avail)
        Pp = 0.0
        umin = avail[0] if avail else None
        for c in range(NCHUNK):
            ap = abs(Pp)
            u = 0.0 if ap == 0.0 else 2.0 ** (np.floor(np.log2(ap)) - 23)
            if umin is None or u < umin:
                Pp += fallback[c, k]
            else:
                uu = None
                for x in reversed(avail):
                    if x <= u:
                        uu = x
                        break
                if uu is None:
                    uu = umin
                Pp += tabs[uu][c, k]
        return Pp

    s_rep = np.array([replay(k, sig_s, s_ab, AV_S.get(k, [])) for k in range(4)])
    sq_rep = np.array([replay(k, sig_q, sq_ab, AV_Q[k]) for k in range(4)])

    c_rep = np.minimum(cnt_ab.sum(0), 2.0**24)
    levels = np.array(lv, np.float64)
    safe = np.maximum(c_rep, 1.0)
    mean = s_rep / safe
    var = sq_rep / safe - mean * mean
    total_mse = np.sum(np.where(c_rep > 0, (mean - levels) ** 2, 0.0))
    total_var = np.sum(np.where(c_rep >= 2, var, 0.0))
    loss = total_mse + total_var

    N = float(N_TOTAL)
    sum_absd = (-dev["T1"].sum() - 2 * float(a) * N
                + 2 * (dev["Bm2a"].sum() - dev["B1"].sum() + dev["Bm1a"].sum()
                       - dev["B2"].sum() + dev["B0"].sum() - dev["B3"].sum()
                       + dev["B1a"].sum()))
    s_ex = s_ab.sum(0)
    c_ex = cnt_ab.sum(0)
    sum_w_wq = sum(float(lv[k]) * s_ex[k] for k in range(4))
    sum_wq2 = sum(float(lv[k])**2 * c_ex[k] for k in range(4))
    sum_d2 = dev["T2"].sum() - 2 * sum_w_wq + sum_wq2

    return np.array([loss, total_mse, total_var, sum_d2 / N, sum_absd / N],
                    np.float32)



# revision 7
# speedup vs baseline: 1.3119x; 1.3119x over previous
"""Trainium2 Bass kernel for nn_BinRegularizer (histogram_binning).

Strategy (v2, host-scheduled quantized sums):
The reference's f32 sequential segment_sum quantizes each element's
contribution to the current accumulator ulp u.  Instead of computing
per-chunk tables on a static ladder of dyadic grids (baseline: ~53
full-tile ops), the host predicts each bin-accumulator's binade
trajectory from a 1/1024 subsample (self-consistent iteration over
per-binade gain estimates), and the device computes per-chunk masked
quantized sums at just TWO grids per (bin, edge): the predicted binade
u_hat(c) and a directional neighbor (covers prediction error of +-1
binade near crossings).  Per-chunk constants ride in [P,1] scalar
operands (chunks == partition rows).  The host replay then picks
between the two columns data-dependently, reproducing the reference
trajectory chunk-by-chunk.

Diagnostics (mean |w-wq|, mean (w-wq)^2) come from a fused d-route:
idx = round(clip(w/a)) via magic-add, d = w - a*idx, ACT Abs/Square
accumulations -- replacing the baseline's 11 exact-stat ops.

Self-contained: hardcodes shapes (4096x16384 f32 weights, alpha[1]),
8 NeuronCores, contiguous 8M-element shards per core.
"""
import sys

sys.path.insert(0, "/opt/trn_rl_repo")

import numpy as np

f32 = np.float32

P = 128
F = 2048
NT = 32
NCORES = 8
CORE_ELEMS = P * F * NT
N_TOTAL = CORE_ELEMS * NCORES
NCHUNK = NCORES * NT * P          # 32768 chunks of 2048, stream order
CORE_CHUNKS = NT * P

LG_EXACT = -40                    # grids below any f32 ulp: qz == identity
SUBSTRIDE = 1024

# schedule names: (kind, bins, edges). s-side on w, q-side on z=w|w|.
SCHEDS = ["s01", "s3", "q0", "q1", "q2", "q3"]

# emits: name -> (sched, y-kind, edge, side). side: 'lo' (sum below edge)
# or 'hi' (sum above edge). edge is the threshold id.
EMITS = [
    ("es0", "s01", "th1", "lo"),
    ("es1", "s01", "th2", "lo"),
    ("es3", "s3", "th3", "hi"),
    ("eq0", "q0", "m1", "lo"),
    ("eq1m1", "q1", "m1", "lo"),
    ("eq1m2", "q1", "m2", "lo"),
    ("eq2", "q2", "p2", "lo"),    # fold: on |z| = w^2 grid
    ("eq3", "q3", "p2", "hi"),
]
# emits computed on DVE as stt(y, ST, zeros, sub, min) (value = sum min(y-st,0));
# the rest on ACT as Relu(bias +- y) with accum (value = sum relu(st-y) = -min-sum
# for 'lo' with scale=-1, or sum relu(y-st) for 'hi' with scale=+1, bias=-st).
DVE_EMITS = {"es0"}
POOL_EMITS = set()

NG_SLOTS = 8
# per-tile const columns: 12 M cols (6 scheds x 2 widths) + 16 bias cols
CT_COLS = 28

_CACHE = {}


def MS(u):
    return f32(f32(3.0 * 2.0**22) * f32(u))


def _qz_of(x, u):
    m = MS(u)
    return f32(f32(f32(x) + m) - m)


def _build_program():
    import concourse.bacc as bacc
    import concourse.tile as tile
    from concourse import mybir

    AL = mybir.AluOpType
    AF = mybir.ActivationFunctionType
    DT = mybir.dt.float32

    # const slot bookkeeping
    gslot = {"RINV": 0, "NEGA": 1, "TH1": 2, "TH2": 3, "TH3": 4}
    tcol = {}

    def tc_(name):
        if name not in tcol:
            tcol[name] = len(tcol)
        return tcol[name]

    for sc in SCHEDS:
        tc_(f"M_{sc}_a")
        tc_(f"M_{sc}_b")
    for nm, sc, edge, side in EMITS:
        tc_(f"B_{nm}_a")
        tc_(f"B_{nm}_b")
    assert len(tcol) <= CT_COLS, len(tcol)

    layout_d = {}
    layout_a = {}
    layout_p = {}
    for nm in ("C1", "C2", "C3"):
        layout_d[nm] = len(layout_d)
    for nm, sc, edge, side in EMITS:
        for wd in ("a", "b"):
            full = f"{nm}_{wd}"
            if nm in DVE_EMITS:
                layout_d[full] = len(layout_d)
            elif nm in POOL_EMITS:
                layout_p[full] = len(layout_p)
            else:
                layout_a[full] = len(layout_a)
    layout_a["SAD"] = len(layout_a)
    layout_a["SSQ"] = len(layout_a)
    NQ_D = len(layout_d)
    NQ_A = len(layout_a)
    NQ_P = len(layout_p)

    nc = bacc.Bacc("TRN2", target_bir_lowering=False, debug=False,
                   num_devices=NCORES)
    W = nc.dram_tensor("w", [NT * P, F], DT, kind="ExternalInput")
    CG = nc.dram_tensor("cg", [P, NG_SLOTS], DT, kind="ExternalInput")
    CT = nc.dram_tensor("ct", [P, CT_COLS * NT], DT, kind="ExternalInput")
    OUTD = nc.dram_tensor("outd", [P, NQ_D * NT], DT, kind="ExternalOutput")
    OUTA = nc.dram_tensor("outa", [P, NQ_A * NT], DT, kind="ExternalOutput")
    OUTP = (nc.dram_tensor("outp", [P, NQ_P * NT], DT,
                            kind="ExternalOutput") if NQ_P else None)
    Wv = W[:, :].rearrange("(t p) f -> t p f", p=P)

    with tile.TileContext(nc) as tc:
        with tc.tile_pool(name="wp", bufs=3) as wpool, \
             tc.tile_pool(name="yp", bufs=4) as ypool, \
             tc.tile_pool(name="zp", bufs=2) as zpool, \
             tc.tile_pool(name="dp", bufs=4) as dpool, \
             tc.tile_pool(name="singles", bufs=1) as singles:
            cgd = singles.tile([P, NG_SLOTS], DT)
            cga = singles.tile([P, NG_SLOTS], DT)
            ctd = singles.tile([P, CT_COLS * NT], DT)
            cta = singles.tile([P, CT_COLS * NT], DT)
            zeros = singles.tile([P, F], DT)
            std = singles.tile([P, NQ_D * NT], DT)
            sta = singles.tile([P, NQ_A * NT], DT)
            stp = singles.tile([P, NQ_P * NT], DT) if NQ_P else None
            gd = singles.tile([P, F], DT)
            ga = singles.tile([P, F], DT)
            gp = singles.tile([P, F], DT) if NQ_P else None

            nc.sync.dma_start(out=cgd, in_=CG[:, :])
            nc.sync.dma_start(out=ctd, in_=CT[:, :])
            nc.scalar.copy(out=cga, in_=cgd)
            nc.scalar.copy(out=cta, in_=ctd)
            nc.vector.memset(zeros, 0.0)

            def g_d(nm):
                return cgd[:, gslot[nm]:gslot[nm] + 1]

            def ct_d(nm, t):
                j = tcol[nm]
                return ctd[:, j * NT + t:j * NT + t + 1]

            def ct_a(nm, t):
                j = tcol[nm]
                return cta[:, j * NT + t:j * NT + t + 1]

            def st(name, t):
                if name in layout_d:
                    q = layout_d[name]
                    return std[:, q * NT + t:q * NT + t + 1]
                if name in layout_p:
                    q = layout_p[name]
                    return stp[:, q * NT + t:q * NT + t + 1]
                q = layout_a[name]
                return sta[:, q * NT + t:q * NT + t + 1]

            for t in range(NT):
                w = wpool.tile([P, F], DT, tag="w")
                nc.sync.dma_start(out=w, in_=Wv[t])

                # counts
                for nm, cn, op in (("C1", "TH1", AL.is_gt),
                                   ("C2", "TH2", AL.is_ge),
                                   ("C3", "TH3", AL.is_gt)):
                    nc.vector.tensor_scalar(
                        out=gd[:, :], in0=w[:, :], scalar1=g_d(cn),
                        scalar2=None, op0=op, op1=AL.add,
                        accum_out=st(nm, t))

                # d-route: idx = round(clip(w/a, -2, 1)); d = w - a*idx
                tt = dpool.tile([P, F], DT, tag="d")
                nc.vector.tensor_scalar(
                    out=tt[:, :], in0=w[:, :], scalar1=g_d("RINV"),
                    scalar2=-2.0, op0=AL.mult, op1=AL.max)
                im = dpool.tile([P, F], DT, tag="d")
                nc.vector.tensor_scalar(
                    out=im[:, :], in0=tt[:, :], scalar1=1.0,
                    scalar2=12582912.0, op0=AL.min, op1=AL.add)
                idx = dpool.tile([P, F], DT, tag="d")
                nc.vector.tensor_scalar(
                    out=idx[:, :], in0=im[:, :], scalar1=12582912.0,
                    scalar2=None, op0=AL.subtract)
                dt_ = dpool.tile([P, F], DT, tag="d")
                nc.vector.scalar_tensor_tensor(
                    out=dt_[:, :], in0=idx[:, :], scalar=g_d("NEGA"),
                    in1=w[:, :], op0=AL.mult, op1=AL.add)
                nc.scalar.activation(out=ga[:, :], in_=dt_[:, :], func=AF.Abs,
                                     bias=0.0, scale=1.0,
                                     accum_out=st("SAD", t))
                nc.scalar.activation(out=ga[:, :], in_=dt_[:, :],
                                     func=AF.Square, bias=0.0, scale=1.0,
                                     accum_out=st("SSQ", t))

                # z = w * |w|; azt = |z| = w*w (exact: same rne magnitude)
                absw = zpool.tile([P, F], DT, tag="absw")
                nc.vector.scalar_tensor_tensor(
                    out=absw[:, :], in0=w[:, :], scalar=-1.0,
                    in1=w[:, :], op0=AL.mult, op1=AL.max)
                z = zpool.tile([P, F], DT, tag="z")
                nc.vector.tensor_mul(out=z[:, :], in0=w[:, :], in1=absw[:, :])
                azt = zpool.tile([P, F], DT, tag="azt")
                nc.scalar.activation(out=azt[:, :], in_=w[:, :],
                                     func=AF.Square, bias=0.0, scale=1.0)

                # y tiles per (sched, width)
                ytiles = {}
                for sc in SCHEDS:
                    ysrc = w if sc.startswith("s") else (azt if sc == "q2"
                                                         else z)
                    for wd in ("a", "b"):
                        y = ypool.tile([P, F], DT, tag="y")
                        nc.vector.tensor_scalar(
                            out=y[:, :], in0=ysrc[:, :],
                            scalar1=ct_d(f"M_{sc}_{wd}", t),
                            scalar2=None, op0=AL.add)
                        ytiles[(sc, wd)] = y

                # emits
                for nm, sc, edge, side in EMITS:
                    for wd in ("a", "b"):
                        y = ytiles[(sc, wd)]
                        full = f"{nm}_{wd}"
                        if nm in DVE_EMITS or nm in POOL_EMITS:
                            eng = (nc.vector if nm in DVE_EMITS
                                   else nc.gpsimd)
                            gout = gd if nm in DVE_EMITS else gp
                            # acc = sum min/max(y - st, 0)  (st stored as +ST)
                            eng.scalar_tensor_tensor(
                                out=gout[:, :], in0=y[:, :],
                                scalar=ct_d(f"B_{nm}_{wd}", t),
                                in1=zeros[:, :], op0=AL.subtract,
                                op1=(AL.min if side == "lo" else AL.max),
                                accum_out=st(full, t))
                        elif side == "lo":
                            # acc = sum relu(st - y)  (bias col stores +ST)
                            nc.scalar.activation(
                                out=ga[:, :], in_=y[:, :], func=AF.Relu,
                                bias=ct_a(f"B_{nm}_{wd}", t), scale=-1.0,
                                accum_out=st(full, t))
                        else:
                            # acc = sum relu(y - st)  (bias col stores -ST)
                            nc.scalar.activation(
                                out=ga[:, :], in_=y[:, :], func=AF.Relu,
                                bias=ct_a(f"B_{nm}_{wd}", t), scale=1.0,
                                accum_out=st(full, t))

            nc.sync.dma_start(out=OUTD[:, :], in_=std)
            nc.sync.dma_start(out=OUTA[:, :], in_=sta)
            if NQ_P:
                nc.sync.dma_start(out=OUTP[:, :], in_=stp)

    nc.compile()
    return nc, gslot, tcol, layout_d, layout_a, layout_p


def _get_program():
    if "prog" not in _CACHE:
        _CACHE["prog"] = _build_program()
    return _CACHE["prog"]


# ---------------- host scheduling ----------------

def _thresholds(a):
    return (f32(f32(-1.5) * a), f32(f32(-0.5) * a), f32(f32(0.5) * a))


def _predict_lgs(w_full, a):
    """Per-chunk predicted accumulator binade (log2 ulp) per (kind, bin).
    Self-consistent iteration over subsample-estimated per-binade gains."""
    th1, th2, th3 = _thresholds(a)
    sub = w_full[::SUBSTRIDE]
    c1 = sub > th1
    c2 = sub >= th2
    c3 = sub > th3
    kidx = c1.astype(np.int8) + c2.astype(np.int8) + c3.astype(np.int8)
    zsub = (sub * np.abs(sub)).astype(f32)

    preds = {}
    for kind in ("s", "q"):
        x_all = sub if kind == "s" else np.abs(zsub)
        for k in ((0, 1, 3) if kind == "s" else (0, 1, 2, 3)):
            xk = x_all[kidx == k]
            frac = float((kidx == k).mean())
            g_exact = float(xk.astype(np.float64).mean()) * frac * F
            gains = {}
            for e in range(-30, -1):
                m = MS(2.0 ** e)
                q = ((xk.astype(f32) + m) - m).astype(np.float64)
                gains[e] = float(q.mean()) * frac * F
            Pacc = 0.0
            lgs = np.empty(NCHUNK, np.int64)
            for c in range(NCHUNK):
                ap = abs(Pacc)
                lg = LG_EXACT if ap == 0.0 else max(
                    int(np.floor(np.log2(ap))) - 23, LG_EXACT)
                lgs[c] = lg
                if lg < -30:
                    Pacc += g_exact
                else:
                    Pacc += gains[min(lg, -2)]
            preds[(kind, k)] = lgs
    return preds


def _directional(lgs):
    """Second-column binade per chunk: +1 within 8% of segment length before
    each predicted upward crossing, else -1."""
    lg2 = lgs - 1
    # find crossing indices (where lgs increases)
    cross = np.nonzero(np.diff(lgs) > 0)[0] + 1  # first index of new binade
    starts = np.concatenate([[0], cross])
    ends = np.concatenate([cross, [NCHUNK]])
    for s, e in zip(starts, ends):
        if e < NCHUNK:
            wwin = max(int(0.08 * (e - s)), 16)
            lo = max(s, e - wwin)
            lg2[lo:e] = lgs[lo:e] + 1
    return lg2


def _sched_grids(w_full, a):
    preds = _predict_lgs(w_full, a)
    grids = {}
    for key, lgs in preds.items():
        grids[key] = (lgs, _directional(lgs))
    return grids


def kernel(weights, alpha):
    from concourse.bass_utils import run_bass_kernel_spmd

    w_full = np.ascontiguousarray(weights, dtype=np.float32).reshape(-1)
    a = f32(np.asarray(alpha, dtype=np.float32).reshape(-1)[0])
    assert w_full.size == N_TOTAL

    nc, gslot, tcol, layout_d, layout_a, layout_p = _get_program()
    th1, th2, th3 = _thresholds(a)
    tau1 = f32(th1 * th1)
    tau2 = f32(th3 * th3)

    grids = _sched_grids(w_full, a)
    # map sched name -> (kind, bin)
    sched_key = {"s01": ("s", 0), "s3": ("s", 3),
                 "q0": ("q", 0), "q1": ("q", 1), "q2": ("q", 2),
                 "q3": ("q", 3)}
    edge_th = {"th1": th1, "th2": th2, "th3": th3,
               "m1": f32(-tau1), "m2": f32(-tau2), "p2": tau2}

    # per-chunk u arrays and const columns
    ucols = {}    # (sched, wd) -> u per chunk (float64)
    ccols = {}    # colname -> per-chunk f32 value
    for sc in SCHEDS:
        lgs_a, lgs_b = grids[sched_key[sc]]
        for wd, lgs in (("a", lgs_a), ("b", lgs_b)):
            u = np.exp2(lgs.astype(np.float64))
            ucols[(sc, wd)] = u
            ccols[f"M_{sc}_{wd}"] = MS(np.exp2(lgs.astype(f32)))
    for nm, sc, edge, side in EMITS:
        t = edge_th[edge]
        for wd in ("a", "b"):
            u = ucols[(sc, wd)].astype(f32)
            m = (f32(3.0 * 2.0**22) * u).astype(f32)
            qt = ((f32(t) + m) - m).astype(f32)
            stv = (m + qt).astype(f32)
            on_act = nm not in DVE_EMITS and nm not in POOL_EMITS
            ccols[f"B_{nm}_{wd}"] = (-stv if (side == "hi" and on_act)
                                     else stv)

    gvals = np.zeros(NG_SLOTS, f32)
    gvals[gslot["RINV"]] = f32(1.0) / a
    gvals[gslot["NEGA"]] = -a
    gvals[gslot["TH1"]] = th1
    gvals[gslot["TH2"]] = th2
    gvals[gslot["TH3"]] = th3
    cg_np = np.ascontiguousarray(np.broadcast_to(gvals, (P, NG_SLOTS)))

    in_maps = []
    for c in range(NCORES):
        shard = w_full[c * CORE_ELEMS:(c + 1) * CORE_ELEMS].reshape(NT * P, F)
        ct = np.zeros((P, CT_COLS * NT), f32)
        sl = slice(c * CORE_CHUNKS, (c + 1) * CORE_CHUNKS)
        for nm, j in tcol.items():
            ct[:, j * NT:(j + 1) * NT] = ccols[nm][sl].reshape(NT, P).T
        in_maps.append({"w": shard, "cg": cg_np,
                        "ct": np.ascontiguousarray(ct)})

    res = run_bass_kernel_spmd(nc, in_maps, core_ids=list(range(NCORES)))

    dev = {}
    for lay, key in ((layout_d, "outd"), (layout_a, "outa"),
                     (layout_p, "outp")):
        nq = len(lay)
        if nq == 0:
            continue
        for qname, qi in lay.items():
            arr = np.empty(NCHUNK, np.float64)
            for c in range(NCORES):
                block = res.results[c][key].reshape(P, nq, NT)
                arr[c * CORE_CHUNKS:(c + 1) * CORE_CHUNKS] = \
                    block[:, qi, :].T.reshape(-1)
            dev[qname] = arr

    return _finish(dev, ucols, a)


def _acc_lo(dev, nm, wd):
    """sum min(y-st,0) per chunk from device accums."""
    v = dev[f"{nm}_{wd}"]
    return v if (nm in DVE_EMITS or nm in POOL_EMITS) else -v


def _finish(dev, ucols, a):
    th1, th2, th3 = _thresholds(a)
    tau1 = f32(th1 * th1)
    tau2 = f32(th3 * th3)
    lv = [f32(f32(-2) * a), f32(f32(-1) * a), f32(0.0), f32(f32(1) * a)]
    n = float(F)

    C1, C2, C3 = dev["C1"], dev["C2"], dev["C3"]

    def qt_col(sc, wd, t):
        u = ucols[(sc, wd)].astype(f32)
        m = (f32(3.0 * 2.0**22) * u).astype(f32)
        return (((f32(t) + m) - m)).astype(np.float64)

    # per-chunk per-width bin columns
    cols_s = {}
    cols_q = {}
    for wd in ("a", "b"):
        p_s0 = _acc_lo(dev, "es0", wd) + qt_col("s01", wd, th1) * (n - C1)
        p_s1t2 = _acc_lo(dev, "es1", wd) + qt_col("s01", wd, th2) * (n - C2)
        s3 = dev[f"es3_{wd}"] + qt_col("s3", wd, th3) * C3
        cols_s[(0, wd)] = p_s0
        cols_s[(1, wd)] = p_s1t2 - p_s0
        cols_s[(3, wd)] = s3

        q0 = -(_acc_lo(dev, "eq0", wd) + qt_col("q0", wd, f32(-tau1)) * (n - C1))
        q1m1 = _acc_lo(dev, "eq1m1", wd) + qt_col("q1", wd, f32(-tau1)) * (n - C1)
        q1m2 = _acc_lo(dev, "eq1m2", wd) + qt_col("q1", wd, f32(-tau2)) * (n - C2)
        q2 = _acc_lo(dev, "eq2", wd) + qt_col("q2", wd, tau2) * (C2 - C3)
        q3 = dev[f"eq3_{wd}"] + qt_col("q3", wd, tau2) * C3
        cols_q[(0, wd)] = q0
        cols_q[(1, wd)] = -(q1m2 - q1m1)
        cols_q[(2, wd)] = q2
        cols_q[(3, wd)] = q3

    sched_of = {("s", 0): "s01", ("s", 1): "s01", ("s", 3): "s3",
                ("q", 0): "q0", ("q", 1): "q1", ("q", 2): "q2",
                ("q", 3): "q3"}

    def replay(kind, k, cols):
        sc = sched_of[(kind, k)]
        lga = np.log2(ucols[(sc, "a")]).astype(np.int64)
        lgb = np.log2(ucols[(sc, "b")]).astype(np.int64)
        ca, cb = cols[(k, "a")], cols[(k, "b")]
        Pacc = 0.0
        for c in range(NCHUNK):
            ap = abs(Pacc)
            lg = -200 if ap == 0.0 else int(np.floor(np.log2(ap))) - 23
            la, lb = lga[c], lgb[c]
            # pick the available column closest to the true binade
            if abs(lg - la) <= abs(lg - lb):
                Pacc += ca[c]
            else:
                Pacc += cb[c]
        return Pacc

    s_rep = np.zeros(4)
    sq_rep = np.zeros(4)
    for k in (0, 1, 3):
        s_rep[k] = replay("s", k, cols_s)
    s_rep[2] = 0.0
    for k in range(4):
        sq_rep[k] = replay("q", k, cols_q)

    cnt_tot = np.stack([n * NCHUNK - C1.sum(), (C1 - C2).sum(),
                        (C2 - C3).sum(), C3.sum()])
    c_rep = np.minimum(cnt_tot, 2.0**24)
    levels = np.array(lv, np.float64)
    safe = np.maximum(c_rep, 1.0)
    mean = s_rep / safe
    var = sq_rep / safe - mean * mean
    total_mse = np.sum(np.where(c_rep > 0, (mean - levels) ** 2, 0.0))
    total_var = np.sum(np.where(c_rep >= 2, var, 0.0))
    loss = total_mse + total_var

    N = float(N_TOTAL)
    mean_distance = dev["SAD"].sum() / N
    quantization_mse = dev["SSQ"].sum() / N

    return np.array([loss, total_mse, total_var, quantization_mse,
                     mean_distance], np.float32)


# revision 8
# speedup vs baseline: 13.0414x; 9.9405x over previous
"""Trainium2 Bass kernel for nn_BinRegularizer (histogram_binning).

Strategy (v2, host-scheduled quantized sums):
The reference's f32 sequential segment_sum quantizes each element's
contribution to the current accumulator ulp u.  Instead of computing
per-chunk tables on a static ladder of dyadic grids (baseline: ~53
full-tile ops), the host predicts each bin-accumulator's binade
trajectory from a 1/1024 subsample (self-consistent iteration over
per-binade gain estimates), and the device computes per-chunk masked
quantized sums at just TWO grids per (bin, edge): the predicted binade
u_hat(c) and a directional neighbor (covers prediction error of +-1
binade near crossings).  Per-chunk constants ride in [P,1] scalar
operands (chunks == partition rows).  The host replay then picks
between the two columns data-dependently, reproducing the reference
trajectory chunk-by-chunk.

Diagnostics (mean |w-wq|, mean (w-wq)^2) come from a fused d-route:
idx = round(clip(w/a)) via magic-add, d = w - a*idx, ACT Abs/Square
accumulations -- replacing the baseline's 11 exact-stat ops.

Self-contained: hardcodes shapes (4096x16384 f32 weights, alpha[1]),
8 NeuronCores, contiguous 8M-element shards per core.
"""
import sys

sys.path.insert(0, "/opt/trn_rl_repo")

import numpy as np

f32 = np.float32

P = 128
F = 2048
NT = 32
NCORES = 8
CORE_ELEMS = P * F * NT
N_TOTAL = CORE_ELEMS * NCORES
NCHUNK = NCORES * NT * P          # 32768 chunks of 2048, stream order
CORE_CHUNKS = NT * P

LG_EXACT = -40                    # grids below any f32 ulp: qz == identity
SUBSTRIDE = 1024

# schedule names: (kind, bins, edges). s-side on w, q-side on z=w|w|.
SCHEDS = ["s01", "s3", "q03", "q1", "q2"]

# emits: name -> (sched, y-kind, edge, side). side: 'lo' (sum below edge)
# or 'hi' (sum above edge). edge is the threshold id.
EMITS = [
    ("es0", "s01", "th1", "lo"),
    ("es1", "s01", "th2", "lo"),
    ("es3", "s3", "th3", "hi"),
    ("eq0", "q03", "m1", "lo"),
    ("eq1m1", "q1", "m1", "lo"),
    ("eq1m2", "q1", "m2", "lo"),
    ("eq2", "q2", "p2", "lo"),    # fold: on |z| = w^2 grid
    ("eq3", "q03", "p2", "hi"),
]
# emits computed on DVE as stt(y, ST, zeros, sub, min) (value = sum min(y-st,0));
# the rest on ACT as Relu(bias +- y) with accum (value = sum relu(st-y) = -min-sum
# for 'lo' with scale=-1, or sum relu(y-st) for 'hi' with scale=+1, bias=-st).
DVE_EMITS = {"es0", "es1"}
POOL_EMITS = set()

NG_SLOTS = 8
# per-tile const columns: 10 M cols (5 scheds x 2 widths) + 16 bias cols
CT_COLS = 26

_CACHE = {}


def MS(u):
    return f32(f32(3.0 * 2.0**22) * f32(u))


def _qz_of(x, u):
    m = MS(u)
    return f32(f32(f32(x) + m) - m)


def _build_program():
    import concourse.bacc as bacc
    import concourse.tile as tile
    from concourse import mybir

    AL = mybir.AluOpType
    AF = mybir.ActivationFunctionType
    DT = mybir.dt.float32

    # const slot bookkeeping
    gslot = {"RINV": 0, "NEGA": 1, "TH1": 2, "TH2": 3, "TH3": 4}
    tcol = {}

    def tc_(name):
        if name not in tcol:
            tcol[name] = len(tcol)
        return tcol[name]

    for sc in SCHEDS:
        tc_(f"M_{sc}_a")
        tc_(f"M_{sc}_b")
    for nm, sc, edge, side in EMITS:
        tc_(f"B_{nm}_a")
        tc_(f"B_{nm}_b")
    assert len(tcol) <= CT_COLS, len(tcol)

    layout_d = {}
    layout_a = {}
    layout_p = {}
    for nm in ("C1", "C2", "C3"):
        layout_d[nm] = len(layout_d)
    for nm, sc, edge, side in EMITS:
        for wd in ("a", "b"):
            full = f"{nm}_{wd}"
            if nm in DVE_EMITS:
                layout_d[full] = len(layout_d)
            elif nm in POOL_EMITS:
                layout_p[full] = len(layout_p)
            else:
                layout_a[full] = len(layout_a)
    layout_a["SAD"] = len(layout_a)
    layout_a["SSQ"] = len(layout_a)
    NQ_D = len(layout_d)
    NQ_A = len(layout_a)
    NQ_P = len(layout_p)

    nc = bacc.Bacc("TRN2", target_bir_lowering=False, debug=False,
                   num_devices=NCORES)
    W = nc.dram_tensor("w", [NT * P, F], DT, kind="ExternalInput")
    CG = nc.dram_tensor("cg", [P, NG_SLOTS], DT, kind="ExternalInput")
    CT = nc.dram_tensor("ct", [P, CT_COLS * NT], DT, kind="ExternalInput")
    OUTD = nc.dram_tensor("outd", [P, NQ_D * NT], DT, kind="ExternalOutput")
    OUTA = nc.dram_tensor("outa", [P, NQ_A * NT], DT, kind="ExternalOutput")
    OUTP = (nc.dram_tensor("outp", [P, NQ_P * NT], DT,
                            kind="ExternalOutput") if NQ_P else None)
    Wv = W[:, :].rearrange("(t p) f -> t p f", p=P)

    with tile.TileContext(nc) as tc:
        with tc.tile_pool(name="wp", bufs=3) as wpool, \
             tc.tile_pool(name="yp", bufs=4) as ypool, \
             tc.tile_pool(name="zp", bufs=2) as zpool, \
             tc.tile_pool(name="dp", bufs=4) as dpool, \
             tc.tile_pool(name="singles", bufs=1) as singles:
            cgd = singles.tile([P, NG_SLOTS], DT)
            cga = singles.tile([P, NG_SLOTS], DT)
            ctd = singles.tile([P, CT_COLS * NT], DT)
            cta = singles.tile([P, CT_COLS * NT], DT)
            zeros = singles.tile([P, F], DT)
            std = singles.tile([P, NQ_D * NT], DT)
            sta = singles.tile([P, NQ_A * NT], DT)
            stp = singles.tile([P, NQ_P * NT], DT) if NQ_P else None
            gd = singles.tile([P, F], DT)
            ga = singles.tile([P, F], DT)
            gp = singles.tile([P, F], DT) if NQ_P else None

            nc.sync.dma_start(out=cgd, in_=CG[:, :])
            nc.sync.dma_start(out=ctd, in_=CT[:, :])
            nc.scalar.copy(out=cga, in_=cgd)
            nc.scalar.copy(out=cta, in_=ctd)
            nc.vector.memset(zeros, 0.0)

            def g_d(nm):
                return cgd[:, gslot[nm]:gslot[nm] + 1]

            def ct_d(nm, t):
                j = tcol[nm]
                return ctd[:, j * NT + t:j * NT + t + 1]

            def ct_a(nm, t):
                j = tcol[nm]
                return cta[:, j * NT + t:j * NT + t + 1]

            def st(name, t):
                if name in layout_d:
                    q = layout_d[name]
                    return std[:, q * NT + t:q * NT + t + 1]
                if name in layout_p:
                    q = layout_p[name]
                    return stp[:, q * NT + t:q * NT + t + 1]
                q = layout_a[name]
                return sta[:, q * NT + t:q * NT + t + 1]

            for t in range(NT):
                w = wpool.tile([P, F], DT, tag="w")
                nc.sync.dma_start(out=w, in_=Wv[t])

                # counts
                for nm, cn, op in (("C1", "TH1", AL.is_gt),
                                   ("C2", "TH2", AL.is_ge),
                                   ("C3", "TH3", AL.is_gt)):
                    nc.vector.tensor_scalar(
                        out=gd[:, :], in0=w[:, :], scalar1=g_d(cn),
                        scalar2=None, op0=op, op1=AL.add,
                        accum_out=st(nm, t))

                # d-route: idx = round(clip(w/a, -2, 1)); d = w - a*idx
                tt = dpool.tile([P, F], DT, tag="d")
                nc.vector.tensor_scalar(
                    out=tt[:, :], in0=w[:, :], scalar1=g_d("RINV"),
                    scalar2=-2.0, op0=AL.mult, op1=AL.max)
                im = dpool.tile([P, F], DT, tag="d")
                nc.vector.tensor_scalar(
                    out=im[:, :], in0=tt[:, :], scalar1=1.0,
                    scalar2=12582912.0, op0=AL.min, op1=AL.add)
                idx = dpool.tile([P, F], DT, tag="d")
                nc.vector.tensor_scalar(
                    out=idx[:, :], in0=im[:, :], scalar1=12582912.0,
                    scalar2=None, op0=AL.subtract)
                dt_ = dpool.tile([P, F], DT, tag="d")
                nc.vector.scalar_tensor_tensor(
                    out=dt_[:, :], in0=idx[:, :], scalar=g_d("NEGA"),
                    in1=w[:, :], op0=AL.mult, op1=AL.add)
                nc.scalar.activation(out=ga[:, :], in_=dt_[:, :], func=AF.Abs,
                                     bias=0.0, scale=1.0,
                                     accum_out=st("SAD", t))
                nc.scalar.activation(out=ga[:, :], in_=dt_[:, :],
                                     func=AF.Square, bias=0.0, scale=1.0,
                                     accum_out=st("SSQ", t))

                # z = w * |w|; azt = |z| = w*w (exact: same rne magnitude)
                absw = zpool.tile([P, F], DT, tag="absw")
                nc.vector.scalar_tensor_tensor(
                    out=absw[:, :], in0=w[:, :], scalar=-1.0,
                    in1=w[:, :], op0=AL.mult, op1=AL.max)
                z = zpool.tile([P, F], DT, tag="z")
                nc.vector.tensor_mul(out=z[:, :], in0=w[:, :], in1=absw[:, :])
                azt = zpool.tile([P, F], DT, tag="azt")
                nc.scalar.activation(out=azt[:, :], in_=w[:, :],
                                     func=AF.Square, bias=0.0, scale=1.0)

                # y tiles per (sched, width)
                ytiles = {}
                for sc in SCHEDS:
                    ysrc = w if sc.startswith("s") else (azt if sc == "q2"
                                                         else z)
                    for wd in ("a", "b"):
                        y = ypool.tile([P, F], DT, tag="y")
                        nc.vector.tensor_scalar(
                            out=y[:, :], in0=ysrc[:, :],
                            scalar1=ct_d(f"M_{sc}_{wd}", t),
                            scalar2=None, op0=AL.add)
                        ytiles[(sc, wd)] = y

                # emits
                for nm, sc, edge, side in EMITS:
                    for wd in ("a", "b"):
                        y = ytiles[(sc, wd)]
                        full = f"{nm}_{wd}"
                        if nm in DVE_EMITS or nm in POOL_EMITS:
                            eng = (nc.vector if nm in DVE_EMITS
                                   else nc.gpsimd)
                            gout = gd if nm in DVE_EMITS else gp
                            # acc = sum min/max(y - st, 0)  (st stored as +ST)
                            eng.scalar_tensor_tensor(
                                out=gout[:, :], in0=y[:, :],
                                scalar=ct_d(f"B_{nm}_{wd}", t),
                                in1=zeros[:, :], op0=AL.subtract,
                                op1=(AL.min if side == "lo" else AL.max),
                                accum_out=st(full, t))
                        elif side == "lo":
                            # acc = sum relu(st - y)  (bias col stores +ST)
                            nc.scalar.activation(
                                out=ga[:, :], in_=y[:, :], func=AF.Relu,
                                bias=ct_a(f"B_{nm}_{wd}", t), scale=-1.0,
                                accum_out=st(full, t))
                        else:
                            # acc = sum relu(y - st)  (bias col stores -ST)
                            nc.scalar.activation(
                                out=ga[:, :], in_=y[:, :], func=AF.Relu,
                                bias=ct_a(f"B_{nm}_{wd}", t), scale=1.0,
                                accum_out=st(full, t))

            nc.sync.dma_start(out=OUTD[:, :], in_=std)
            nc.sync.dma_start(out=OUTA[:, :], in_=sta)
            if NQ_P:
                nc.sync.dma_start(out=OUTP[:, :], in_=stp)

    nc.compile()
    return nc, gslot, tcol, layout_d, layout_a, layout_p


def _get_program():
    if "prog" not in _CACHE:
        _CACHE["prog"] = _build_program()
    return _CACHE["prog"]


# ---------------- host scheduling ----------------

def _thresholds(a):
    return (f32(f32(-1.5) * a), f32(f32(-0.5) * a), f32(f32(0.5) * a))


def _predict_lgs(w_full, a):
    """Per-chunk predicted accumulator binade (log2 ulp) per (kind, bin).
    Self-consistent iteration over subsample-estimated per-binade gains."""
    th1, th2, th3 = _thresholds(a)
    sub = w_full[::SUBSTRIDE]
    c1 = sub > th1
    c2 = sub >= th2
    c3 = sub > th3
    kidx = c1.astype(np.int8) + c2.astype(np.int8) + c3.astype(np.int8)
    zsub = (sub * np.abs(sub)).astype(f32)

    preds = {}
    for kind in ("s", "q"):
        x_all = sub if kind == "s" else np.abs(zsub)
        for k in ((0, 1, 3) if kind == "s" else (0, 1, 2, 3)):
            xk = x_all[kidx == k]
            frac = float((kidx == k).mean())
            g_exact = float(xk.astype(np.float64).mean()) * frac * F
            gains = {}
            for e in range(-30, -1):
                m = MS(2.0 ** e)
                q = ((xk.astype(f32) + m) - m).astype(np.float64)
                gains[e] = float(q.mean()) * frac * F
            Pacc = 0.0
            lgs = np.empty(NCHUNK, np.int64)
            for c in range(NCHUNK):
                ap = abs(Pacc)
                lg = LG_EXACT if ap == 0.0 else max(
                    int(np.floor(np.log2(ap))) - 23, LG_EXACT)
                lgs[c] = lg
                if lg < -30:
                    Pacc += g_exact
                else:
                    Pacc += gains[min(lg, -2)]
            preds[(kind, k)] = lgs
    return preds


def _directional(lgs):
    """Second-column binade per chunk: +1 within 8% of segment length before
    each predicted upward crossing, else -1."""
    lg2 = lgs - 1
    # find crossing indices (where lgs increases)
    cross = np.nonzero(np.diff(lgs) > 0)[0] + 1  # first index of new binade
    starts = np.concatenate([[0], cross])
    ends = np.concatenate([cross, [NCHUNK]])
    for s, e in zip(starts, ends):
        if e < NCHUNK:
            wwin = max(int(0.08 * (e - s)), 16)
            lo = max(s, e - wwin)
            lg2[lo:e] = lgs[lo:e] + 1
    return lg2


def _sched_grids(w_full, a):
    preds = _predict_lgs(w_full, a)
    grids = {}
    for key, lgs in preds.items():
        grids[key] = (lgs, _directional(lgs))
    return grids


def kernel(weights, alpha):
    from concourse.bass_utils import run_bass_kernel_spmd

    w_full = np.ascontiguousarray(weights, dtype=np.float32).reshape(-1)
    a = f32(np.asarray(alpha, dtype=np.float32).reshape(-1)[0])
    assert w_full.size == N_TOTAL

    nc, gslot, tcol, layout_d, layout_a, layout_p = _get_program()
    th1, th2, th3 = _thresholds(a)
    tau1 = f32(th1 * th1)
    tau2 = f32(th3 * th3)

    grids = _sched_grids(w_full, a)
    # map sched name -> (kind, bin)
    sched_key = {"s01": ("s", 0), "s3": ("s", 3),
                 "q03": ("q", 3), "q1": ("q", 1), "q2": ("q", 2)}
    edge_th = {"th1": th1, "th2": th2, "th3": th3,
               "m1": f32(-tau1), "m2": f32(-tau2), "p2": tau2}

    # per-chunk u arrays and const columns
    ucols = {}    # (sched, wd) -> u per chunk (float64)
    ccols = {}    # colname -> per-chunk f32 value
    for sc in SCHEDS:
        lgs_a, lgs_b = grids[sched_key[sc]]
        for wd, lgs in (("a", lgs_a), ("b", lgs_b)):
            u = np.exp2(lgs.astype(np.float64))
            ucols[(sc, wd)] = u
            ccols[f"M_{sc}_{wd}"] = MS(np.exp2(lgs.astype(f32)))
    for nm, sc, edge, side in EMITS:
        t = edge_th[edge]
        for wd in ("a", "b"):
            u = ucols[(sc, wd)].astype(f32)
            m = (f32(3.0 * 2.0**22) * u).astype(f32)
            qt = ((f32(t) + m) - m).astype(f32)
            stv = (m + qt).astype(f32)
            on_act = nm not in DVE_EMITS and nm not in POOL_EMITS
            ccols[f"B_{nm}_{wd}"] = (-stv if (side == "hi" and on_act)
                                     else stv)

    gvals = np.zeros(NG_SLOTS, f32)
    gvals[gslot["RINV"]] = f32(1.0) / a
    gvals[gslot["NEGA"]] = -a
    gvals[gslot["TH1"]] = th1
    gvals[gslot["TH2"]] = th2
    gvals[gslot["TH3"]] = th3
    cg_np = np.ascontiguousarray(np.broadcast_to(gvals, (P, NG_SLOTS)))

    in_maps = []
    for c in range(NCORES):
        shard = w_full[c * CORE_ELEMS:(c + 1) * CORE_ELEMS].reshape(NT * P, F)
        ct = np.zeros((P, CT_COLS * NT), f32)
        sl = slice(c * CORE_CHUNKS, (c + 1) * CORE_CHUNKS)
        for nm, j in tcol.items():
            ct[:, j * NT:(j + 1) * NT] = ccols[nm][sl].reshape(NT, P).T
        in_maps.append({"w": shard, "cg": cg_np,
                        "ct": np.ascontiguousarray(ct)})

    res = run_bass_kernel_spmd(nc, in_maps, core_ids=list(range(NCORES)))

    dev = {}
    for lay, key in ((layout_d, "outd"), (layout_a, "outa"),
                     (layout_p, "outp")):
        nq = len(lay)
        if nq == 0:
            continue
        for qname, qi in lay.items():
            arr = np.empty(NCHUNK, np.float64)
            for c in range(NCORES):
                block = res.results[c][key].reshape(P, nq, NT)
                arr[c * CORE_CHUNKS:(c + 1) * CORE_CHUNKS] = \
                    block[:, qi, :].T.reshape(-1)
            dev[qname] = arr

    return _finish(dev, ucols, a)


def _acc_lo(dev, nm, wd):
    """sum min(y-st,0) per chunk from device accums."""
    v = dev[f"{nm}_{wd}"]
    return v if (nm in DVE_EMITS or nm in POOL_EMITS) else -v


def _finish(dev, ucols, a):
    th1, th2, th3 = _thresholds(a)
    tau1 = f32(th1 * th1)
    tau2 = f32(th3 * th3)
    lv = [f32(f32(-2) * a), f32(f32(-1) * a), f32(0.0), f32(f32(1) * a)]
    n = float(F)

    C1, C2, C3 = dev["C1"], dev["C2"], dev["C3"]

    def qt_col(sc, wd, t):
        u = ucols[(sc, wd)].astype(f32)
        m = (f32(3.0 * 2.0**22) * u).astype(f32)
        return (((f32(t) + m) - m)).astype(np.float64)

    # per-chunk per-width bin columns
    cols_s = {}
    cols_q = {}
    for wd in ("a", "b"):
        p_s0 = _acc_lo(dev, "es0", wd) + qt_col("s01", wd, th1) * (n - C1)
        p_s1t2 = _acc_lo(dev, "es1", wd) + qt_col("s01", wd, th2) * (n - C2)
        s3 = dev[f"es3_{wd}"] + qt_col("s3", wd, th3) * C3
        cols_s[(0, wd)] = p_s0
        cols_s[(1, wd)] = p_s1t2 - p_s0
        cols_s[(3, wd)] = s3

        q0 = -(_acc_lo(dev, "eq0", wd) + qt_col("q03", wd, f32(-tau1)) * (n - C1))
        q1m1 = _acc_lo(dev, "eq1m1", wd) + qt_col("q1", wd, f32(-tau1)) * (n - C1)
        q1m2 = _acc_lo(dev, "eq1m2", wd) + qt_col("q1", wd, f32(-tau2)) * (n - C2)
        q2 = _acc_lo(dev, "eq2", wd) + qt_col("q2", wd, tau2) * (C2 - C3)
        q3 = dev[f"eq3_{wd}"] + qt_col("q03", wd, tau2) * C3
        cols_q[(0, wd)] = q0
        cols_q[(1, wd)] = -(q1m2 - q1m1)
        cols_q[(2, wd)] = q2
        cols_q[(3, wd)] = q3

    sched_of = {("s", 0): "s01", ("s", 1): "s01", ("s", 3): "s3",
                ("q", 0): "q03", ("q", 1): "q1", ("q", 2): "q2",
                ("q", 3): "q03"}

    def replay(kind, k, cols):
        sc = sched_of[(kind, k)]
        lga = np.log2(ucols[(sc, "a")]).astype(np.int64)
        lgb = np.log2(ucols[(sc, "b")]).astype(np.int64)
        ca, cb = cols[(k, "a")], cols[(k, "b")]
        Pacc = 0.0
        for c in range(NCHUNK):
            ap = abs(Pacc)
            lg = -200 if ap == 0.0 else int(np.floor(np.log2(ap))) - 23
            la, lb = lga[c], lgb[c]
            # pick the available column closest to the true binade
            if abs(lg - la) <= abs(lg - lb):
                Pacc += ca[c]
            else:
                Pacc += cb[c]
        return Pacc

    s_rep = np.zeros(4)
    sq_rep = np.zeros(4)
    for k in (0, 1, 3):
        s_rep[k] = replay("s", k, cols_s)
    s_rep[2] = 0.0
    for k in range(4):
        sq_rep[k] = replay("q", k, cols_q)

    cnt_tot = np.stack([n * NCHUNK - C1.sum(), (C1 - C2).sum(),
                        (C2 - C3).sum(), C3.sum()])
    c_rep = np.minimum(cnt_tot, 2.0**24)
    levels = np.array(lv, np.float64)
    safe = np.maximum(c_rep, 1.0)
    mean = s_rep / safe
    var = sq_rep / safe - mean * mean
    total_mse = np.sum(np.where(c_rep > 0, (mean - levels) ** 2, 0.0))
    total_var = np.sum(np.where(c_rep >= 2, var, 0.0))
    loss = total_mse + total_var

    N = float(N_TOTAL)
    mean_distance = dev["SAD"].sum() / N
    quantization_mse = dev["SSQ"].sum() / N

    return np.array([loss, total_mse, total_var, quantization_mse,
                     mean_distance], np.float32)
